# revision 21
# baseline (speedup 1.0000x reference)
"""TRN2 Bass kernel for nn_BaseQuantizer (VQ codebook quantizer).

Data-parallel over 8 NeuronCores: x [16,4096,64] sharded along batch
(2 batches/core), codebook [64,1024] replicated. Per core:
  - scores[t,k] = x.cb - 0.5*||c_k||^2 via 3-term bf16 hi/lo matmul
    (exact: 0 argmin flips vs fp32 on this data regime)
  - argmax: DVE reduce-max -> m; ACT relu(2^27*(m-s)) (exact 0 at the
    argmax by power-of-2 scaling + Sterbenz); DVE adds iota (f16 2x)
    and a fused tensor_scalar min-accum (4x) extracts the index
  - dequantize: batched indirect-DMA gather from transposed codebook
  - histogram: 32x32 factorized one-hot matmul (idx = 32q + r)
  - loss partial: sum(||x||^2) - 2*sum(max_score)
Host combines shards and computes the EMA cluster frequency like the
reference does.
"""

import numpy as np

DIM = 64
NCODES = 1024
B, L = 16, 4096
NCORES = 8
BPC = B // NCORES          # batches per core
TOK = BPC * L              # tokens per core
P = 128
NTILES = TOK // P
NSEG = 32                  # histogram factorization: idx = 32*q + r
BIG = float(2 ** 27)       # relu scale; exact for pow2, 2^27*min_gap >> 1023

_CACHE = {}


def _build_nc():
    from concourse import bacc, mybir, tile
    import concourse.bass as bass_mod
    from concourse.masks import make_identity

    F32 = mybir.dt.float32
    F16 = mybir.dt.float16
    BF16 = mybir.dt.bfloat16
    U32 = mybir.dt.uint32
    X = mybir.AxisListType.X
    Alu = mybir.AluOpType
    Act = mybir.ActivationFunctionType

    nc = bacc.Bacc("TRN2", target_bir_lowering=False, debug=False)
    x_hbm = nc.dram_tensor("x", [TOK, DIM], F32, kind="ExternalInput")
    cb_hbm = nc.dram_tensor("cb", [DIM, NCODES], F32, kind="ExternalInput")
    xq_hbm = nc.dram_tensor("xq", [P, NTILES * DIM], F32, kind="ExternalOutput")
    idx_hbm = nc.dram_tensor("idx", [P, NTILES], U32, kind="ExternalOutput")
    hist_hbm = nc.dram_tensor("hist", [NSEG, NSEG], F32, kind="ExternalOutput")
    loss_hbm = nc.dram_tensor("loss", [1, 1], F32, kind="ExternalOutput")
    cbT_dram = nc.dram_tensor("cbT_scratch", [NCODES, DIM], F32)

    with tile.TileContext(nc) as tc, \
         tc.tile_pool(name="const", bufs=1) as cpool, \
         tc.tile_pool(name="work", bufs=3) as wpool, \
         tc.tile_pool(name="acc", bufs=1) as apool, \
         tc.tile_pool(name="ps", bufs=2, space="PSUM") as pspool, \
         tc.tile_pool(name="ps1", bufs=1, space="PSUM") as ps1pool:

        # ---------------- setup ----------------
        id_bf = cpool.tile([P, P], BF16)
        make_identity(nc, id_bf[:])
        id_f32 = cpool.tile([P, P], F32)
        make_identity(nc, id_f32[:])

        cb_sb = cpool.tile([DIM, NCODES], F32)
        nc.sync.dma_start(cb_sb[:], cb_hbm[:])

        # bf16 hi part of codebook, stacked twice along contraction for the
        # K=128 matmul with [xh; xl].
        chch = cpool.tile([P, NCODES], BF16)
        nc.vector.tensor_copy(chch[0:DIM, :], cb_sb[:])
        nc.sync.dma_start(chch[DIM:P, :], chch[0:DIM, :])

        # bf16 lo part + 3 exact-bf16-decomposed bias rows (-0.5*||c||^2).
        cl_aug = cpool.tile([DIM + 3, NCODES], BF16)
        nc.vector.tensor_tensor(cl_aug[0:DIM, :], cb_sb[:], chch[0:DIM, :],
                                Alu.subtract)

        sq = cpool.tile([DIM, NCODES], F32)
        nc.scalar.activation(sq[:], cb_sb[:], Act.Square)
        ones64 = cpool.tile([DIM, 1], F32)
        nc.vector.memset(ones64[:], 1.0)
        h_sb = cpool.tile([1, NCODES], F32)
        for half in range(2):
            cbsq_ps = ps1pool.tile([1, 512], F32, tag="misc")
            nc.tensor.matmul(cbsq_ps[:], ones64[:], sq[:, half * 512:(half + 1) * 512],
                             start=True, stop=True)
            nc.scalar.activation(h_sb[:, half * 512:(half + 1) * 512], cbsq_ps[:],
                                 Act.Copy, scale=-0.5)
        hh = cpool.tile([1, NCODES], BF16)
        nc.vector.tensor_copy(hh[:], h_sb[:])
        d1 = cpool.tile([1, NCODES], F32)
        nc.vector.tensor_tensor(d1[:], h_sb[:], hh[:], Alu.subtract)
        hl = cpool.tile([1, NCODES], BF16)
        nc.vector.tensor_copy(hl[:], d1[:])
        d2 = cpool.tile([1, NCODES], F32)
        nc.vector.tensor_tensor(d2[:], d1[:], hl[:], Alu.subtract)
        hll = cpool.tile([1, NCODES], BF16)
        nc.vector.tensor_copy(hll[:], d2[:])
        nc.sync.dma_start(cl_aug[DIM + 0:DIM + 1, :], hh[:])
        nc.sync.dma_start(cl_aug[DIM + 1:DIM + 2, :], hl[:])
        nc.sync.dma_start(cl_aug[DIM + 2:DIM + 3, :], hll[:])

        # transposed codebook in DRAM for the dequantize gather
        for c in range(NCODES // P):
            cbT_ps = pspool.tile([P, DIM], F32, tag="ps_T", bufs=2)
            nc.tensor.transpose(cbT_ps[:], cb_sb[:, c * P:(c + 1) * P],
                                id_f32[0:DIM, 0:DIM])
            cbT_stage = wpool.tile([P, DIM], F32, tag="cbT_stage")
            nc.scalar.activation(cbT_stage[:], cbT_ps[:], Act.Copy)
            nc.sync.dma_start(cbT_dram[c * P:(c + 1) * P, :], cbT_stage[:])

        # iota constants
        iota_i = cpool.tile([P, NCODES], mybir.dt.int32)
        nc.gpsimd.iota(iota_i[:], pattern=[[1, NCODES]], channel_multiplier=0)
        iota16 = cpool.tile([P, NCODES], F16)
        nc.vector.tensor_copy(iota16[:], iota_i[:])
        iota_i2 = cpool.tile([P, NTILES * NSEG], mybir.dt.int32)
        nc.gpsimd.iota(iota_i2[:], pattern=[[0, NTILES], [1, NSEG]],
                       channel_multiplier=0)
        iota_f2 = cpool.tile([P, NTILES * NSEG], F32)
        nc.vector.tensor_copy(iota_f2[:], iota_i2[:])

        ones_col = cpool.tile([P, 1], F32)
        nc.vector.memset(ones_col[:], 1.0)

        # E stationaries: fixed 3-rotation, ones rows set once
        E_rot = [apool.tile([DIM + 3, P], BF16, name=f"E{i}", tag=f"E{i}")
                 for i in range(3)]
        for E in E_rot:
            nc.gpsimd.memset(E[DIM:DIM + 3, :], 1.0)

        # accumulators
        m_all = apool.tile([P, NTILES], F32)
        xsq_all = apool.tile([P, NTILES], F32)
        idxf_all = apool.tile([P, NTILES], F32)

        # ---------------- main loop ----------------
        for g in range(NTILES):
            xt = wpool.tile([P, DIM], F32, tag="xt")
            nc.sync.dma_start(xt[:], x_hbm[g * P:(g + 1) * P, :])

            xhl = wpool.tile([P, P], BF16, tag="xhl")
            nc.gpsimd.tensor_copy(xhl[:, 0:DIM], xt[:])
            nc.gpsimd.tensor_tensor(xhl[:, DIM:P], xt[:], xhl[:, 0:DIM],
                                    Alu.subtract)

            ps_T = pspool.tile([P, P], BF16, tag="ps_T", bufs=2)
            nc.tensor.transpose(ps_T[0:DIM, :], xhl[:, 0:DIM], id_bf[:])
            nc.tensor.transpose(ps_T[DIM:P, :], xhl[:, DIM:P], id_bf[:])
            D = wpool.tile([P, P], BF16, tag="D")
            nc.scalar.activation(D[:], ps_T[:], Act.Copy)
            E = E_rot[g % 3]
            nc.scalar.activation(E[0:DIM, :], ps_T[0:DIM, :], Act.Copy)

            ps_s = pspool.tile([P, NCODES], F32, tag="ps_s", bufs=2)
            nc.tensor.matmul(ps_s[:, 0:512], D[:], chch[:, 0:512],
                             start=True, stop=False)
            nc.tensor.matmul(ps_s[:, 512:1024], D[:], chch[:, 512:1024],
                             start=True, stop=False)
            nc.tensor.matmul(ps_s[:, 0:512], E[:], cl_aug[:, 0:512],
                             start=False, stop=True)
            nc.tensor.matmul(ps_s[:, 512:1024], E[:], cl_aug[:, 512:1024],
                             start=False, stop=True)

            # pass 1: exact fp32 max (written straight into the accumulator)
            nc.vector.tensor_reduce(m_all[:, g:g + 1], ps_s[:], X, Alu.max)

            # m_big = 2^27 * m (exact, power-of-two scale)
            m_big = wpool.tile([P, 1], F32, tag="m_big")
            nc.gpsimd.tensor_scalar(m_big[:], m_all[:, g:g + 1], BIG, None,
                                    Alu.mult)

            # pass 2 (ACT): tmp = relu(2^27*(m - s)); == 0 exactly at argmax
            tmp = wpool.tile([P, NCODES], F16, tag="tmp")
            nc.scalar.activation(tmp[:], ps_s[:], Act.Relu, bias=m_big[:, 0:1],
                                 scale=-BIG)

            # keys = tmp + iota (f16, 2x); fused min-accum extracts index (4x)
            keys = wpool.tile([P, NCODES], F16, tag="keys")
            nc.vector.tensor_tensor(keys[:], tmp[:], iota16[:], Alu.add)
            junk2 = wpool.tile([P, NCODES], F16, tag="junk2")
            nc.vector.tensor_scalar(junk2[:], keys[:], 60000.0, None, Alu.min,
                                    Alu.min, accum_out=idxf_all[:, g:g + 1])

            # dequantize: gather codebook rows by index
            idx_col = wpool.tile([P, 1], U32, tag="idx_col")
            nc.gpsimd.tensor_copy(idx_col[:], idxf_all[:, g:g + 1])
            xq_sb = wpool.tile([P, DIM], F32, tag="xq_sb")
            nc.gpsimd.indirect_dma_start(
                out=xq_sb[:],
                out_offset=None,
                in_=cbT_dram[:, :],
                in_offset=bass_mod.IndirectOffsetOnAxis(ap=idx_col[:, 0:1],
                                                        axis=0),
            )
            nc.sync.dma_start(xq_hbm[:, g * DIM:(g + 1) * DIM], xq_sb[:])

            # ||x||^2 per token (for the loss)
            junk = wpool.tile([P, DIM], F32, tag="junk")
            nc.scalar.activation(junk[:], xt[:], Act.Square,
                                 accum_out=xsq_all[:, g:g + 1])

        # ---------------- end phase ----------------
        idx_u32 = apool.tile([P, NTILES], U32)
        nc.vector.tensor_copy(idx_u32[:], idxf_all[:])
        nc.sync.dma_start(idx_hbm[:], idx_u32[:])

        # histogram: idx = 32*q + r
        q_u = apool.tile([P, NTILES], U32)
        nc.vector.tensor_scalar(q_u[:], idx_u32[:], 5, None,
                                Alu.logical_shift_right)
        r_u = apool.tile([P, NTILES], U32)
        nc.vector.tensor_scalar(r_u[:], idx_u32[:], 31, None, Alu.bitwise_and)
        q_f = apool.tile([P, NTILES], F32)
        nc.vector.tensor_copy(q_f[:], q_u[:])
        r_f = apool.tile([P, NTILES], F32)
        nc.vector.tensor_copy(r_f[:], r_u[:])

        oh_q = apool.tile([P, NTILES, NSEG], BF16)
        nc.vector.tensor_tensor(
            oh_q[:], iota_f2[:].rearrange("p (g s) -> p g s", s=NSEG),
            q_f[:].to_broadcast([P, NTILES, NSEG]), Alu.is_equal)
        oh_r = apool.tile([P, NTILES, NSEG], BF16)
        nc.vector.tensor_tensor(
            oh_r[:], iota_f2[:].rearrange("p (g s) -> p g s", s=NSEG),
            r_f[:].to_broadcast([P, NTILES, NSEG]), Alu.is_equal)

        hist_ps = ps1pool.tile([NSEG, NSEG], F32, tag="misc")
        for g in range(NTILES):
            nc.tensor.matmul(hist_ps[:], oh_q[:, g, :], oh_r[:, g, :],
                             start=(g == 0), stop=(g == NTILES - 1))
        hist_sb = apool.tile([NSEG, NSEG], F32)
        nc.scalar.activation(hist_sb[:], hist_ps[:], Act.Copy)
        nc.sync.dma_start(hist_hbm[:], hist_sb[:])

        # loss partial: sum_t (||x_t||^2 - 2*max_score_t)
        s1 = apool.tile([P, 1], F32)
        nc.vector.tensor_reduce(s1[:], xsq_all[:], X, Alu.add)
        s2 = apool.tile([P, 1], F32)
        nc.vector.tensor_reduce(s2[:], m_all[:], X, Alu.add)
        nc.vector.tensor_scalar(s2[:], s2[:], -2.0, None, Alu.mult)
        comb = apool.tile([P, 1], F32)
        nc.vector.tensor_tensor(comb[:], s1[:], s2[:], Alu.add)
        loss_ps = ps1pool.tile([1, 1], F32, tag="misc")
        nc.tensor.matmul(loss_ps[:], comb[:], ones_col[:], start=True, stop=True)
        loss_sb = apool.tile([1, 1], F32)
        nc.scalar.activation(loss_sb[:], loss_ps[:], Act.Copy)
        nc.sync.dma_start(loss_hbm[:], loss_sb[:])

    nc.compile()
    return nc


def _get_nc():
    if "nc" not in _CACHE:
        _CACHE["nc"] = _build_nc()
    return _CACHE["nc"]


def kernel(x, codebook, cluster_frequency, _trace=False):
    from concourse.bass_utils import run_bass_kernel_spmd

    nc = _get_nc()
    x32 = np.ascontiguousarray(np.asarray(x), dtype=np.float32)
    cb32 = np.ascontiguousarray(np.asarray(codebook), dtype=np.float32)
    cf32 = np.asarray(cluster_frequency, dtype=np.float32)

    in_maps = [
        {"x": x32[c * BPC:(c + 1) * BPC].reshape(TOK, DIM), "cb": cb32}
        for c in range(NCORES)
    ]
    res = run_bass_kernel_spmd(nc, in_maps, list(range(NCORES)), trace=_trace)
    rs = res.results

    xq = np.concatenate(
        [r["xq"].reshape(P, NTILES, DIM).transpose(1, 0, 2).reshape(BPC, L, DIM)
         for r in rs], axis=0)
    idx = np.concatenate(
        [r["idx"].T.reshape(BPC, L) for r in rs], axis=0).astype(np.int32)
    count = np.zeros(NCODES, dtype=np.float32)
    for r in rs:
        count += r["hist"].reshape(-1)
    loss_sum = np.float32(0.0)
    for r in rs:
        loss_sum = np.float32(loss_sum + np.float32(r["loss"][0, 0]))

    n_elem = np.float32(x32.size)
    m = np.float32(loss_sum / n_elem)
    inner_loss = np.float32(m + m)

    alpha = np.float32(0.95)
    ncf = (alpha * cf32 + (np.float32(1.0) - alpha) * count).astype(np.float32)

    x_q_st = x32 + (xq - x32)

    if _trace:
        _CACHE["last_result"] = res
    return x_q_st, idx, inner_loss, ncf
